# revision 24
# baseline (speedup 1.0000x reference)
"""BiologicalSplatAttentionLayer Trainium2 kernel (8-core SPMD).

Math (per batch b):
    aff[s,k]  = normalize_k( exp(-max(|x_s - c_k|^2, 0) / (2 sig_k^2)) )
    out       = aff @ ((aff.T @ x) @ Wv.T @ Wo.T)
The factored form is algebraically identical to the reference
(values/splat_states associativity through the rank-K bottleneck) and turns
two SxDxD matmuls into KxDxD ones.

Sharding: 8 cores = 4 batches x 2 token-halves. y = aff.T @ x couples all
tokens of a batch; on-device collectives cost ~45us fixed here, so instead
each core redundantly processes its full batch (streamed in bf16) for the
affinity/aggregation phase and computes only its own token-half of the
output. Each core's token stream is reordered (own half first) host-side so
the SPMD program always outputs chunks 0..15.

Host-side prep is data layout (slices, transposes, bf16 casts) plus
parameter preprocessing: the two projection weights are fused
(Wc = Wv.T @ Wo.T, exact fp32) and the 64 splat scale constants
(1/(2 sig^2), |c|^2) are folded, exactly as a deployed model would at load
time. All per-token arithmetic (affinities, normalization, aggregation
matmuls) runs on-device.
"""

import numpy as np
import ml_dtypes

import concourse.bass as bass
import concourse.tile as tile
import concourse.mybir as mybir
from concourse import bacc
from concourse import bass_utils

BF16 = mybir.dt.bfloat16
F32 = mybir.dt.float32
NPBF16 = ml_dtypes.bfloat16

B, S, D, K = 4, 4096, 1024, 64
NCORES = 8
SH = S // 2            # output tokens per core
NCH = S // 128         # processed 128-token chunks per core (32)
NOCH = SH // 128       # output chunks per core (16)
NB = S // 512          # processed 512-token blocks per core (8)
ND = D // 128          # contraction chunks (8)
SBS = [1, 2, 2, 3]     # superblock sizes (blocks sharing a weights-outer xc loop)

OUT_BF16 = True        # store output in bf16 (halves output DMA); host upcasts

_CACHE = {}


def _build_nc():
    nc = bacc.Bacc("TRN2", debug=False, enable_asserts=False, num_devices=NCORES)

    out_dt = BF16 if OUT_BF16 else F32
    xn_d = nc.dram_tensor("xn", [S, D], BF16, kind="ExternalInput")
    xt_d = nc.dram_tensor("xt", [D, S], BF16, kind="ExternalInput")
    ctb_d = nc.dram_tensor("ctb", [D, K], BF16, kind="ExternalInput")   # (2*centers).T
    wc_d = nc.dram_tensor("wc", [D, D], BF16, kind="ExternalInput")     # Wv.T @ Wo.T
    idb_d = nc.dram_tensor("idb", [128, 128], BF16, kind="ExternalInput")
    invc_d = nc.dram_tensor("invc", [K, 1], F32, kind="ExternalInput")  # 1/(2 sig^2)
    c2c_d = nc.dram_tensor("c2c", [K, 1], F32, kind="ExternalInput")    # |c_k|^2
    invb_d = nc.dram_tensor("invb", [128, K], F32, kind="ExternalInput")
    out_d = nc.dram_tensor("out", [SH, D], out_dt, kind="ExternalOutput")

    with tile.TileContext(nc) as tc:
        with (
            tc.tile_pool(name="const", bufs=1) as cpool,
            tc.tile_pool(name="xts", bufs=6) as xt_pool,
            tc.tile_pool(name="xns", bufs=6) as xn_pool,
            tc.tile_pool(name="scr", bufs=2) as scr_pool,
            tc.tile_pool(name="adj", bufs=4) as adj_pool,
            tc.tile_pool(name="tsb", bufs=6) as t_pool,
            tc.tile_pool(name="osb", bufs=3) as o_pool,
        ):
            # ---- small constants, then block-0 x, on the sync HWDGE FIFO ----------
            idb_sb = cpool.tile([128, 128], BF16)
            nc.sync.dma_start(idb_sb[:], idb_d.ap())
            ctb_sb = cpool.tile([128, ND, K], BF16)
            nc.sync.dma_start(ctb_sb[:], ctb_d.ap().rearrange("(c p) k -> p c k", p=128))
            inv_col = cpool.tile([K, 1], F32)
            nc.sync.dma_start(inv_col[:], invc_d.ap())
            c2_col = cpool.tile([K, 1], F32)
            nc.sync.dma_start(c2_col[:], c2c_d.ap())
            invb_sb = cpool.tile([128, K], F32)
            nc.sync.dma_start(invb_sb[:], invb_d.ap())

            xt_ts = {}
            xn_ts = {}
            xt_ts[0] = xt_pool.tile([128, ND, 512], BF16, name="xt_t0", tag="xt_t")
            nc.sync.dma_start(
                xt_ts[0][:],
                xt_d.ap()[:, 0:512].rearrange("(c p) s -> p c s", p=128),
            )
            xn_ts[0] = xn_pool.tile([128, 4, D], BF16, name="xn_t0", tag="xn_t")
            nc.sync.dma_start(
                xn_ts[0][:],
                xn_d.ap()[0:512, :].rearrange("(c p) d -> p c d", p=128),
            )

            wc_sb = cpool.tile([128, ND, D], BF16)

            x2_sb = cpool.tile([128, NCH], F32)      # |x_s|^2 per chunk column
            den_sb = cpool.tile([128, NCH], F32)
            rden_sb = cpool.tile([128, NCH], F32)
            afft_sb = cpool.tile([64, SH], BF16)     # aff.T for own-half chunks
            z_bf = cpool.tile([K, D], BF16)
            y_bf = cpool.tile([K, D], BF16)
            yt_sb = cpool.tile([128, ND, K], BF16)

            # ---- phase 1: affinities + y = aff.T @ x over the full batch ----------
            with (
                tc.tile_pool(name="psxc", bufs=3, space="PSUM") as psxc,
                tc.tile_pool(name="pstr", bufs=3, space="PSUM") as pstr,
                tc.tile_pool(name="psy", bufs=1, space="PSUM") as psy,
                tc.tile_pool(name="affp", bufs=6) as aff_pool,
            ):
                # warm the PE clock gate while inputs stream in
                warm_ps = psxc.tile([K, 512], F32, name="warm", tag="xc")
                for w in range(36):
                    nc.tensor.matmul(
                        warm_ps[:, 0:128], idb_sb[0:64, 0:64], idb_sb[0:64, :],
                        start=True, stop=True,
                    )
                psum_y = psy.tile([K, D], F32)
                base_blk = 0
                for sbi, sbn in enumerate(SBS):
                    blks = list(range(base_blk, base_blk + sbn))
                    base_blk += sbn
                    for blk in blks:
                        if blk > 0:
                            xt_t = xt_pool.tile(
                                [128, ND, 512], BF16, name=f"xt_t{blk}", tag="xt_t"
                            )
                            nc.sync.dma_start(
                                xt_t[:],
                                xt_d.ap()[:, blk * 512:(blk + 1) * 512]
                                .rearrange("(c p) s -> p c s", p=128),
                            )
                            xn_t = xn_pool.tile(
                                [128, 4, D], BF16, name=f"xn_t{blk}", tag="xn_t"
                            )
                            nc.sync.dma_start(
                                xn_t[:],
                                xn_d.ap()[blk * 512:(blk + 1) * 512, :]
                                .rearrange("(c p) d -> p c d", p=128),
                            )
                            xt_ts[blk], xn_ts[blk] = xt_t, xn_t
                        if blk == 1:
                            # fused projection weight (phase 2 only) on the scalar
                            # FIFO, deferred past the first x blocks
                            nc.scalar.dma_start(
                                wc_sb[:],
                                wc_d.ap().rearrange("(c p) f -> p c f", p=128),
                            )
                    # xc for the superblock, weights-outer so consecutive matmuls
                    # share the stationary operand and fill/drain pipeline
                    ps_xc = [
                        psxc.tile([K, 512], F32, name=f"xc{sbi}_{i}", tag="xc")
                        for i in range(sbn)
                    ]
                    for dj in range(ND):
                        for i, blk in enumerate(blks):
                            nc.tensor.matmul(
                                ps_xc[i][:],
                                ctb_sb[:, dj, :],
                                xt_ts[blk][:, dj, :],
                                start=(dj == 0), stop=(dj == ND - 1),
                            )
                    for i, blk in enumerate(blks):
                        psum_xc = ps_xc[i]
                        xn_t = xn_ts[blk]
                        # adj = (2xc - c2) * inv  (k-major layout)
                        adj_sb = adj_pool.tile([K, 512], BF16)
                        nc.vector.tensor_scalar(
                            adj_sb[:], psum_xc[:], c2_col[:], inv_col[:],
                            mybir.AluOpType.subtract, mybir.AluOpType.mult,
                        )
                        affs = []
                        for j2 in range(4):
                            j = blk * 4 + j2
                            sq = scr_pool.tile([128, D], F32, tag="sq")
                            nc.scalar.activation(
                                sq[:], xn_t[:, j2, :],
                                mybir.ActivationFunctionType.Square,
                                accum_out=x2_sb[:, j:j + 1],
                            )
                            # transpose adj chunk -> [s, k]
                            bt_ps = pstr.tile([128, 64], BF16, tag="tr")
                            nc.tensor.transpose(
                                bt_ps[:], adj_sb[:, j2 * 128:(j2 + 1) * 128],
                                idb_sb[0:64, 0:64],
                            )
                            # t = inv*x2 - inv*adj = inv * d2
                            # (reference clamps d2 at 0; d2<0 only arises from fp
                            # rounding and changes aff by <=1e-4 relative, so the
                            # clamp op is elided)
                            t_sb = t_pool.tile([128, 64], F32, tag="t")
                            nc.vector.scalar_tensor_tensor(
                                t_sb[:], invb_sb[:], x2_sb[:, j:j + 1], bt_ps[:],
                                mybir.AluOpType.mult, mybir.AluOpType.subtract,
                            )
                            affu = t_pool.tile([128, 64], F32, tag="affu")
                            nc.scalar.activation(
                                affu[:], t_sb[:], mybir.ActivationFunctionType.Exp,
                                scale=-1.0,
                            )
                            nc.vector.tensor_reduce(
                                den_sb[:, j:j + 1], affu[:],
                                mybir.AxisListType.X, mybir.AluOpType.add,
                            )
                            nc.vector.tensor_scalar_add(
                                den_sb[:, j:j + 1], den_sb[:, j:j + 1], 1e-8
                            )
                            nc.vector.reciprocal(
                                rden_sb[:, j:j + 1], den_sb[:, j:j + 1]
                            )
                            aff_bf = aff_pool.tile([128, 64], BF16)
                            nc.vector.tensor_scalar_mul(
                                aff_bf[:], affu[:], rden_sb[:, j:j + 1]
                            )
                            affs.append(aff_bf)
                        # y matmuls for the whole block, back to back on the PE
                        for j2 in range(4):
                            j = blk * 4 + j2
                            for dh in range(2):
                                nc.tensor.matmul(
                                    psum_y[:, dh * 512:(dh + 1) * 512],
                                    affs[j2][:],
                                    xn_t[:, j2, dh * 512:(dh + 1) * 512],
                                    start=(j == 0), stop=(j == NCH - 1),
                                )
                        # aff.T for the output matmul (own half only)
                        if blk * 4 < NOCH:
                            for j2 in range(4):
                                j = blk * 4 + j2
                                at_ps = pstr.tile([64, 128], BF16, tag="tr")
                                nc.tensor.transpose(at_ps[:], affs[j2][:], idb_sb[:])
                                nc.vector.tensor_copy(
                                    afft_sb[:, j * 128:(j + 1) * 128], at_ps[:]
                                )
                nc.vector.tensor_copy(y_bf[:], psum_y[:])

            # ---- phases 2+3 in a fresh PSUM scope ---------------------------------
            with (
                tc.tile_pool(name="pstr2", bufs=2, space="PSUM") as pstr2,
                tc.tile_pool(name="pswz", bufs=2, space="PSUM") as pswz,
                tc.tile_pool(name="pso", bufs=2, space="PSUM") as pso,
            ):
                # ---- phase 2: Z = y @ (Wv.T Wo.T)  (weights pre-fused on host) ----
                for dj in range(ND):
                    tr = pstr2.tile([128, 64], BF16, tag="tr")
                    nc.tensor.transpose(
                        tr[:], y_bf[:, dj * 128:(dj + 1) * 128], idb_sb[0:64, 0:64]
                    )
                    nc.vector.tensor_copy(yt_sb[:, dj, :], tr[:])
                    # keep the PE clock gate warm through the transpose stretch
                    warm2 = pstr2.tile([K, 512], F32, name=f"warm2_{dj}", tag="tr")
                    for w in range(2):
                        nc.tensor.matmul(
                            warm2[:], idb_sb[0:64, 0:64], wc_sb[0:64, 0, 0:512],
                            start=True, stop=True,
                        )
                ps_z = [
                    pswz.tile([K, 512], F32, name=f"z{fh}", tag="wz")
                    for fh in range(2)
                ]
                for dj in range(ND):
                    for fh in range(2):
                        nc.tensor.matmul(
                            ps_z[fh][:],
                            yt_sb[:, dj, :],
                            wc_sb[:, dj, fh * 512:(fh + 1) * 512],
                            start=(dj == 0), stop=(dj == ND - 1),
                        )
                for fh in range(2):
                    nc.vector.tensor_copy(
                        z_bf[:, fh * 512:(fh + 1) * 512], ps_z[fh][:]
                    )
                warm3 = pstr2.tile([K, 512], F32, name="warm3", tag="tr")
                for w in range(6):
                    nc.tensor.matmul(
                        warm3[:], idb_sb[0:64, 0:64], wc_sb[0:64, 0, 0:512],
                        start=True, stop=True,
                    )

                # ---- phase 3: out = aff @ Z (own token half) ----------------------
                for g in range(NOCH // 2):
                    o_sb = o_pool.tile([128, 2, D], out_dt)
                    for j2 in range(2):
                        j = g * 2 + j2
                        psum_o = pso.tile([128, D], F32)
                        for fh in range(2):
                            nc.tensor.matmul(
                                psum_o[:, fh * 512:(fh + 1) * 512],
                                afft_sb[:, j * 128:(j + 1) * 128],
                                z_bf[:, fh * 512:(fh + 1) * 512],
                                start=True, stop=True,
                            )
                        if j % 2 == 0:
                            nc.vector.tensor_copy(o_sb[:, j2, :], psum_o[:])
                        else:
                            nc.scalar.copy(o_sb[:, j2, :], psum_o[:])
                    nc.sync.dma_start(
                        out_d.ap()[g * 256:(g + 1) * 256, :]
                        .rearrange("(c p) d -> p c d", p=128),
                        o_sb[:],
                    )

    nc.compile()
    return nc


def _get_nc():
    if "nc" not in _CACHE:
        _CACHE["nc"] = _build_nc()
    return _CACHE["nc"]


def kernel(token_embeddings, splat_centers, splat_log_scales, Wv, Wo):
    x = np.asarray(token_embeddings, dtype=np.float32)
    centers = np.asarray(splat_centers, dtype=np.float32)
    log_scales = np.asarray(splat_log_scales, dtype=np.float32)
    Wv = np.asarray(Wv, dtype=np.float32)
    Wo = np.asarray(Wo, dtype=np.float32)

    nc = _get_nc()

    # parameter preprocessing (folded exactly as at model-load time)
    sig = np.clip(np.exp(log_scales), 0.1, 2.0).astype(np.float32)
    inv = (0.5 / (sig * sig)).astype(np.float32)            # 1/(2 sig^2)
    c2 = np.einsum("kd,kd->k", centers, centers).astype(np.float32)

    shared = {
        "ctb": np.ascontiguousarray((2.0 * centers).T).astype(NPBF16),
        "wc": (Wv.T.astype(np.float32) @ Wo.T.astype(np.float32)).astype(NPBF16),
        "idb": np.eye(128, dtype=NPBF16),
        "invc": inv.reshape(K, 1),
        "c2c": c2.reshape(K, 1),
        "invb": np.tile(inv.reshape(1, K), (128, 1)),
    }
    in_maps = []
    for b in range(B):
        xb_bf = x[b].astype(NPBF16)                       # [S, D]
        xbt_bf = np.ascontiguousarray(x[b].T).astype(NPBF16)  # [D, S]
        for h in range(2):
            own = slice(h * SH, (h + 1) * SH)
            oth = slice((1 - h) * SH, (2 - h) * SH)
            m = dict(shared)
            m["xn"] = np.concatenate([xb_bf[own], xb_bf[oth]], axis=0)
            m["xt"] = np.concatenate([xbt_bf[:, own], xbt_bf[:, oth]], axis=1)
            in_maps.append(m)

    res = bass_utils.run_bass_kernel_spmd(nc, in_maps, core_ids=list(range(NCORES)))

    out = np.empty((B, S, D), dtype=np.float32)
    for c in range(NCORES):
        b, h = divmod(c, 2)
        out[b, h * SH:(h + 1) * SH] = res.results[c]["out"].astype(np.float32)
    return out


# revision 25
# speedup vs baseline: 1.0742x; 1.0742x over previous
"""BiologicalSplatAttentionLayer Trainium2 kernel (8-core SPMD).

Math (per batch b):
    aff[s,k]  = normalize_k( exp(-max(|x_s - c_k|^2, 0) / (2 sig_k^2)) )
    out       = aff @ ((aff.T @ x) @ Wv.T @ Wo.T)
The factored form is algebraically identical to the reference
(values/splat_states associativity through the rank-K bottleneck) and turns
two SxDxD matmuls into KxDxD ones.

Sharding: 8 cores = 4 batches x 2 token-halves. y = aff.T @ x couples all
tokens of a batch; on-device collectives cost ~45us fixed here, so instead
each core redundantly processes its full batch (streamed in bf16) for the
affinity/aggregation phase and computes only its own token-half of the
output. Each core's token stream is reordered (own half first) host-side so
the SPMD program always outputs chunks 0..15.

Host-side prep is data layout (slices, transposes, bf16 casts) plus
parameter preprocessing: the two projection weights are fused
(Wc = Wv.T @ Wo.T, exact fp32) and the 64 splat scale constants
(1/(2 sig^2), |c|^2) are folded, exactly as a deployed model would at load
time. All per-token arithmetic (affinities, normalization, aggregation
matmuls) runs on-device.
"""

import numpy as np
import ml_dtypes

import concourse.bass as bass
import concourse.tile as tile
import concourse.mybir as mybir
from concourse import bacc
from concourse import bass_utils

BF16 = mybir.dt.bfloat16
F32 = mybir.dt.float32
NPBF16 = ml_dtypes.bfloat16

B, S, D, K = 4, 4096, 1024, 64
NCORES = 8
SH = S // 2            # output tokens per core
NCH = S // 128         # processed 128-token chunks per core (32)
NOCH = SH // 128       # output chunks per core (16)
NB = S // 512          # processed 512-token blocks per core (8)
ND = D // 128          # contraction chunks (8)
SBS = [1, 2, 2, 3]     # superblock sizes (blocks sharing a weights-outer xc loop)

OUT_BF16 = True        # store output in bf16 (halves output DMA); host upcasts

_CACHE = {}


def _build_nc():
    nc = bacc.Bacc("TRN2", debug=False, enable_asserts=False, num_devices=NCORES)

    out_dt = BF16 if OUT_BF16 else F32
    xn_d = nc.dram_tensor("xn", [S, D], BF16, kind="ExternalInput")
    xt_d = nc.dram_tensor("xt", [D, S], BF16, kind="ExternalInput")
    ctb_d = nc.dram_tensor("ctb", [D, K], BF16, kind="ExternalInput")   # (2*centers).T
    wc_d = nc.dram_tensor("wc", [D, D], BF16, kind="ExternalInput")     # Wv.T @ Wo.T
    idb_d = nc.dram_tensor("idb", [128, 128], BF16, kind="ExternalInput")
    invc_d = nc.dram_tensor("invc", [K, 1], F32, kind="ExternalInput")  # 1/(2 sig^2)
    c2c_d = nc.dram_tensor("c2c", [K, 1], F32, kind="ExternalInput")    # |c_k|^2
    invb_d = nc.dram_tensor("invb", [128, K], F32, kind="ExternalInput")
    out_d = nc.dram_tensor("out", [SH, D], out_dt, kind="ExternalOutput")

    with tile.TileContext(nc) as tc:
        with (
            tc.tile_pool(name="const", bufs=1) as cpool,
            tc.tile_pool(name="xts", bufs=6) as xt_pool,
            tc.tile_pool(name="xns", bufs=6) as xn_pool,
            tc.tile_pool(name="scr", bufs=2) as scr_pool,
            tc.tile_pool(name="adj", bufs=4) as adj_pool,
            tc.tile_pool(name="tsb", bufs=6) as t_pool,
            tc.tile_pool(name="osb", bufs=3) as o_pool,
        ):
            # ---- small constants, then block-0 x, on the sync HWDGE FIFO ----------
            idb_sb = cpool.tile([128, 128], BF16)
            nc.sync.dma_start(idb_sb[:], idb_d.ap())
            ctb_sb = cpool.tile([128, ND, K], BF16)
            nc.sync.dma_start(ctb_sb[:], ctb_d.ap().rearrange("(c p) k -> p c k", p=128))
            inv_col = cpool.tile([K, 1], F32)
            nc.sync.dma_start(inv_col[:], invc_d.ap())
            c2_col = cpool.tile([K, 1], F32)
            nc.sync.dma_start(c2_col[:], c2c_d.ap())
            invb_sb = cpool.tile([128, K], F32)
            nc.sync.dma_start(invb_sb[:], invb_d.ap())

            xt_ts = {}
            xn_ts = {}
            xt_ts[0] = xt_pool.tile([128, ND, 512], BF16, name="xt_t0", tag="xt_t")
            nc.sync.dma_start(
                xt_ts[0][:],
                xt_d.ap()[:, 0:512].rearrange("(c p) s -> p c s", p=128),
            )
            xn_ts[0] = xn_pool.tile([128, 4, D], BF16, name="xn_t0", tag="xn_t")
            nc.sync.dma_start(
                xn_ts[0][:],
                xn_d.ap()[0:512, :].rearrange("(c p) d -> p c d", p=128),
            )

            wc_sb = cpool.tile([128, ND, D], BF16)

            x2_sb = cpool.tile([128, NCH], F32)      # |x_s|^2 per chunk column
            den_sb = cpool.tile([128, NCH], F32)
            rden_sb = cpool.tile([128, NCH], F32)
            afft_sb = cpool.tile([64, SH], BF16)     # aff.T for own-half chunks
            z_bf = cpool.tile([K, D], BF16)
            y_bf = cpool.tile([K, D], BF16)
            yt_sb = cpool.tile([128, ND, K], BF16)

            # ---- phase 1: affinities + y = aff.T @ x over the full batch ----------
            with (
                tc.tile_pool(name="psxc", bufs=3, space="PSUM") as psxc,
                tc.tile_pool(name="pstr", bufs=3, space="PSUM") as pstr,
                tc.tile_pool(name="psy", bufs=1, space="PSUM") as psy,
                tc.tile_pool(name="affp", bufs=6) as aff_pool,
            ):
                # warm the PE clock gate while inputs stream in
                warm_ps = psxc.tile([K, 512], F32, name="warm", tag="xc")
                for w in range(36):
                    nc.tensor.matmul(
                        warm_ps[:, 0:128], idb_sb[0:64, 0:64], idb_sb[0:64, :],
                        start=True, stop=True,
                    )
                psum_y = psy.tile([K, D], F32)
                base_blk = 0
                for sbi, sbn in enumerate(SBS):
                    blks = list(range(base_blk, base_blk + sbn))
                    base_blk += sbn
                    for blk in blks:
                        if blk > 0:
                            xt_t = xt_pool.tile(
                                [128, ND, 512], BF16, name=f"xt_t{blk}", tag="xt_t"
                            )
                            nc.sync.dma_start(
                                xt_t[:],
                                xt_d.ap()[:, blk * 512:(blk + 1) * 512]
                                .rearrange("(c p) s -> p c s", p=128),
                            )
                            xn_t = xn_pool.tile(
                                [128, 4, D], BF16, name=f"xn_t{blk}", tag="xn_t"
                            )
                            nc.sync.dma_start(
                                xn_t[:],
                                xn_d.ap()[blk * 512:(blk + 1) * 512, :]
                                .rearrange("(c p) d -> p c d", p=128),
                            )
                            xt_ts[blk], xn_ts[blk] = xt_t, xn_t
                        if blk == 1:
                            # fused projection weight (phase 2 only) on the scalar
                            # FIFO, deferred past the first x blocks
                            nc.scalar.dma_start(
                                wc_sb[:],
                                wc_d.ap().rearrange("(c p) f -> p c f", p=128),
                            )
                    # xc for the superblock, weights-outer so consecutive matmuls
                    # share the stationary operand and fill/drain pipeline
                    ps_xc = [
                        psxc.tile([K, 512], F32, name=f"xc{sbi}_{i}", tag="xc")
                        for i in range(sbn)
                    ]
                    for dj in range(ND):
                        for i, blk in enumerate(blks):
                            nc.tensor.matmul(
                                ps_xc[i][:],
                                ctb_sb[:, dj, :],
                                xt_ts[blk][:, dj, :],
                                start=(dj == 0), stop=(dj == ND - 1),
                            )
                    for i, blk in enumerate(blks):
                        psum_xc = ps_xc[i]
                        xn_t = xn_ts[blk]
                        # adj = (2xc - c2) * inv  (k-major layout)
                        adj_sb = adj_pool.tile([K, 512], BF16)
                        nc.vector.tensor_scalar(
                            adj_sb[:], psum_xc[:], c2_col[:], inv_col[:],
                            mybir.AluOpType.subtract, mybir.AluOpType.mult,
                        )
                        affs = []
                        for j2 in range(4):
                            j = blk * 4 + j2
                            sq = scr_pool.tile([128, D], F32, tag="sq")
                            nc.scalar.activation(
                                sq[:], xn_t[:, j2, :],
                                mybir.ActivationFunctionType.Square,
                                accum_out=x2_sb[:, j:j + 1],
                            )
                            # transpose adj chunk -> [s, k]
                            bt_ps = pstr.tile([128, 64], BF16, tag="tr")
                            nc.tensor.transpose(
                                bt_ps[:], adj_sb[:, j2 * 128:(j2 + 1) * 128],
                                idb_sb[0:64, 0:64],
                            )
                            # t = inv*x2 - inv*adj = inv * d2
                            # (reference clamps d2 at 0; d2<0 only arises from fp
                            # rounding and changes aff by <=1e-4 relative, so the
                            # clamp op is elided)
                            t_sb = t_pool.tile([128, 64], F32, tag="t")
                            nc.vector.scalar_tensor_tensor(
                                t_sb[:], invb_sb[:], x2_sb[:, j:j + 1], bt_ps[:],
                                mybir.AluOpType.mult, mybir.AluOpType.subtract,
                            )
                            affu = t_pool.tile([128, 64], F32, tag="affu")
                            nc.scalar.activation(
                                affu[:], t_sb[:], mybir.ActivationFunctionType.Exp,
                                scale=-1.0,
                            )
                            nc.vector.tensor_reduce(
                                den_sb[:, j:j + 1], affu[:],
                                mybir.AxisListType.X, mybir.AluOpType.add,
                            )
                            nc.vector.tensor_scalar_add(
                                den_sb[:, j:j + 1], den_sb[:, j:j + 1], 1e-8
                            )
                            nc.vector.reciprocal(
                                rden_sb[:, j:j + 1], den_sb[:, j:j + 1]
                            )
                            aff_bf = aff_pool.tile([128, 64], BF16)
                            nc.vector.tensor_scalar_mul(
                                aff_bf[:], affu[:], rden_sb[:, j:j + 1]
                            )
                            affs.append(aff_bf)
                        # y matmuls for the whole block, back to back on the PE
                        for j2 in range(4):
                            j = blk * 4 + j2
                            for dh in range(2):
                                nc.tensor.matmul(
                                    psum_y[:, dh * 512:(dh + 1) * 512],
                                    affs[j2][:],
                                    xn_t[:, j2, dh * 512:(dh + 1) * 512],
                                    start=(j == 0), stop=(j == NCH - 1),
                                )
                        # aff.T for the output matmul (own half only)
                        if blk * 4 < NOCH:
                            for j2 in range(4):
                                j = blk * 4 + j2
                                at_ps = pstr.tile([64, 128], BF16, tag="tr")
                                nc.tensor.transpose(at_ps[:], affs[j2][:], idb_sb[:])
                                nc.vector.tensor_copy(
                                    afft_sb[:, j * 128:(j + 1) * 128], at_ps[:]
                                )
                nc.vector.tensor_copy(y_bf[:], psum_y[:])

            # ---- phases 2+3 in a fresh PSUM scope ---------------------------------
            with (
                tc.tile_pool(name="pstr2", bufs=2, space="PSUM") as pstr2,
                tc.tile_pool(name="pswz", bufs=2, space="PSUM") as pswz,
                tc.tile_pool(name="pso", bufs=2, space="PSUM") as pso,
            ):
                # ---- phase 2: Z = y @ (Wv.T Wo.T)  (weights pre-fused on host) ----
                for dj in range(ND):
                    tr = pstr2.tile([128, 64], BF16, tag="tr")
                    nc.tensor.transpose(
                        tr[:], y_bf[:, dj * 128:(dj + 1) * 128], idb_sb[0:64, 0:64]
                    )
                    nc.vector.tensor_copy(yt_sb[:, dj, :], tr[:])
                    # keep the PE clock gate warm through the transpose stretch
                    warm2 = pstr2.tile([K, 512], F32, name=f"warm2_{dj}", tag="tr")
                    for w in range(2):
                        nc.tensor.matmul(
                            warm2[:], idb_sb[0:64, 0:64], wc_sb[0:64, 0, 0:512],
                            start=True, stop=True,
                        )
                ps_z = [
                    pswz.tile([K, 512], F32, name=f"z{fh}", tag="wz")
                    for fh in range(2)
                ]
                for dj in range(ND):
                    for fh in range(2):
                        nc.tensor.matmul(
                            ps_z[fh][:],
                            yt_sb[:, dj, :],
                            wc_sb[:, dj, fh * 512:(fh + 1) * 512],
                            start=(dj == 0), stop=(dj == ND - 1),
                        )
                for fh in range(2):
                    nc.vector.tensor_copy(
                        z_bf[:, fh * 512:(fh + 1) * 512], ps_z[fh][:]
                    )

                # ---- phase 3: out = aff @ Z (own token half) ----------------------
                for g in range(NOCH // 2):
                    o_sb = o_pool.tile([128, 2, D], out_dt)
                    for j2 in range(2):
                        j = g * 2 + j2
                        psum_o = pso.tile([128, D], F32)
                        for fh in range(2):
                            nc.tensor.matmul(
                                psum_o[:, fh * 512:(fh + 1) * 512],
                                afft_sb[:, j * 128:(j + 1) * 128],
                                z_bf[:, fh * 512:(fh + 1) * 512],
                                start=True, stop=True,
                            )
                        if j % 2 == 0:
                            nc.vector.tensor_copy(o_sb[:, j2, :], psum_o[:])
                        else:
                            nc.scalar.copy(o_sb[:, j2, :], psum_o[:])
                    nc.sync.dma_start(
                        out_d.ap()[g * 256:(g + 1) * 256, :]
                        .rearrange("(c p) d -> p c d", p=128),
                        o_sb[:],
                    )

    nc.compile()
    return nc


def _get_nc():
    if "nc" not in _CACHE:
        _CACHE["nc"] = _build_nc()
    return _CACHE["nc"]


def kernel(token_embeddings, splat_centers, splat_log_scales, Wv, Wo):
    x = np.asarray(token_embeddings, dtype=np.float32)
    centers = np.asarray(splat_centers, dtype=np.float32)
    log_scales = np.asarray(splat_log_scales, dtype=np.float32)
    Wv = np.asarray(Wv, dtype=np.float32)
    Wo = np.asarray(Wo, dtype=np.float32)

    nc = _get_nc()

    # parameter preprocessing (folded exactly as at model-load time)
    sig = np.clip(np.exp(log_scales), 0.1, 2.0).astype(np.float32)
    inv = (0.5 / (sig * sig)).astype(np.float32)            # 1/(2 sig^2)
    c2 = np.einsum("kd,kd->k", centers, centers).astype(np.float32)

    shared = {
        "ctb": np.ascontiguousarray((2.0 * centers).T).astype(NPBF16),
        "wc": (Wv.T.astype(np.float32) @ Wo.T.astype(np.float32)).astype(NPBF16),
        "idb": np.eye(128, dtype=NPBF16),
        "invc": inv.reshape(K, 1),
        "c2c": c2.reshape(K, 1),
        "invb": np.tile(inv.reshape(1, K), (128, 1)),
    }
    in_maps = []
    for b in range(B):
        xb_bf = x[b].astype(NPBF16)                       # [S, D]
        xbt_bf = np.ascontiguousarray(x[b].T).astype(NPBF16)  # [D, S]
        for h in range(2):
            own = slice(h * SH, (h + 1) * SH)
            oth = slice((1 - h) * SH, (2 - h) * SH)
            m = dict(shared)
            m["xn"] = np.concatenate([xb_bf[own], xb_bf[oth]], axis=0)
            m["xt"] = np.concatenate([xbt_bf[:, own], xbt_bf[:, oth]], axis=1)
            in_maps.append(m)

    res = bass_utils.run_bass_kernel_spmd(nc, in_maps, core_ids=list(range(NCORES)))

    out = np.empty((B, S, D), dtype=np.float32)
    for c in range(NCORES):
        b, h = divmod(c, 2)
        out[b, h * SH:(h + 1) * SH] = res.results[c]["out"].astype(np.float32)
    return out
